# revision 9
# baseline (speedup 1.0000x reference)
"""GCN kernel v5: v4 + software-pipelined epilogue.

The epilogue of superblock sb runs AFTER the one-hots/aggregation of
sb+1 are issued, breaking the per-superblock convoy where DVE's
in-order queue blocked sb+1's one-hots behind epilogue TTs that wait on
the full-superblock PE barrier (which stalled matmuls, and through seg
WAR, the gather engine).

Differences from v3 (kernel.py):
  * Bucket capacities quantized to 32 slots (not 128-slot columns):
    padded gather slots drop ~15% (163.5k -> ~140k incl. segment tail
    pads).  Buckets therefore share 128-slot columns.
  * Matmuls stay full-K at partition base 0 (offset matmuls crash the
    runtime): a column shared by several buckets is consumed by several
    matmuls, each using a one-hot built from a SIDE-MASKED sc chunk
    where foreign slots hold the sentinel, so their one-hot rows are
    all-zero and contribute nothing.
  * One-hot chunks are built per (group, half, range) over that bucket's
    column span; boundary columns are built once per adjacent bucket
    (small duplicated DVE work, ~+15%).
"""

import sys

sys.path.insert(0, "/opt/trn_rl_repo")

import numpy as np

from concourse import bacc, bass, mybir, tile
from concourse import bass_utils

P = 128
D = 64
WN = 128

N_NODES = 100000
N_CORES = 8
NODES_PER_CORE = N_NODES // N_CORES

GW = 256
GROUPS = (NODES_PER_CORE + GW - 1) // GW  # 49
NODES_PAD = GROUPS * GW
SBG = 7
SB = GROUPS // SBG
assert SBG * SB == GROUPS
WINDOWS = 2 * GROUPS
WPB = 2 * SBG

RANGE = 32768
RANGE_STARTS = [0, RANGE, 2 * RANGE, 3 * RANGE]
RANGE_ROWS = [RANGE, RANGE, RANGE, N_NODES - 3 * RANGE]
NR = 4

QUEUE_RATE = [7.84, 16.4, 16.4, 16.4]
Q32 = 32  # bucket slot quantum

F32 = mybir.dt.float32
I16 = mybir.dt.int16


def _layout(caps):
    """Slot/column/chunk layout from per-bucket slot capacities.

    caps: [GROUPS, 2, NR] slot counts (multiples of 32).
    Returns dict with:
      bucket_slot  [GROUPS,2,NR]: global slot offset of bucket start
      seg_slot_off [SB][NR], seg_cols [SB][NR] (gather segments, padded
        to 128 slots)
      chunks: list of (jg, h, r, seg_local_col0, ncols, scg_col0)
        in (sb, r, j, h) order; scg_col0 indexes the side-masked sc
        array (boundary columns duplicated per bucket)
      tot_slots, tot_chunk_cols, max_chunk_cols
    """
    caps = np.asarray(caps).reshape(GROUPS, 2, NR)
    bucket_slot = np.zeros((GROUPS, 2, NR), dtype=np.int64)
    seg_slot_off = [[0] * NR for _ in range(SB)]
    seg_cols = [[0] * NR for _ in range(SB)]
    chunks = []
    slot = 0
    scg_col = 0
    max_chunk = 0
    for sb in range(SB):
        for r in range(NR):
            seg_slot_off[sb][r] = slot
            local = 0
            seg_chunks = []
            for j in range(SBG):
                jg = sb * SBG + j
                for h in range(2):
                    cap = int(caps[jg, h, r])
                    bucket_slot[jg, h, r] = slot + local
                    c0 = local // P
                    c1 = -(-(local + cap) // P)
                    seg_chunks.append((jg, h, r, c0, c1 - c0, scg_col))
                    scg_col += c1 - c0
                    max_chunk = max(max_chunk, c1 - c0)
                    local += cap
            seg_slots = -(-local // P) * P  # pad segment to whole columns
            seg_cols[sb][r] = seg_slots // P
            slot += seg_slots
            chunks.extend(seg_chunks)
    return {
        "caps": caps,
        "bucket_slot": bucket_slot,
        "seg_slot_off": seg_slot_off,
        "seg_cols": seg_cols,
        "chunks": chunks,
        "tot_slots": slot,
        "tot_chunk_cols": scg_col,
        "max_chunk_cols": max_chunk,
    }


def build_program(caps, n_reps=1):
    L = _layout(caps)
    caps = L["caps"]
    tot_slots = L["tot_slots"]
    tot_cc = L["tot_chunk_cols"]
    max_k = L["max_chunk_cols"]
    # chunk lookup by (jg, h, r)
    chunk_of = {}
    for jg, h, r, c0, nck, cc0 in L["chunks"]:
        chunk_of[(jg, h, r)] = (c0, nck, cc0)

    qload = [0.0, 0.0, 0.0, 0.0]
    qassign = {}
    for sb in range(SB):
        for r in range(NR):
            nd = L["seg_cols"][sb][r] * P
            q = min(range(4), key=lambda i: qload[i] + nd * QUEUE_RATE[i])
            qassign[(sb, r)] = q
            qload[q] += nd * QUEUE_RATE[q]

    nc = bacc.Bacc("TRN2", target_bir_lowering=False, debug=False,
                   num_swdge_queues=4, dynamic_dma_scratch_size=32768)

    feat = nc.dram_tensor("feat32", [N_NODES, D], F32, kind="ExternalInput")
    gidx = nc.dram_tensor("gidx", [P, tot_slots // 16], I16, kind="ExternalInput")
    scg = nc.dram_tensor("scg", [P, tot_cc], F32, kind="ExternalInput")
    invdeg = nc.dram_tensor("invdeg", [P, WINDOWS], F32, kind="ExternalInput")
    wgc = nc.dram_tensor("wgc", [D, 3], F32, kind="ExternalInput")
    wlint = nc.dram_tensor("wlint", [D, D], F32, kind="ExternalInput")
    bgc_rep = nc.dram_tensor("bgc_rep", [P, 3 * WPB], F32, kind="ExternalInput")
    blin_rep = nc.dram_tensor("blin_rep", [P, D * SBG], F32, kind="ExternalInput")
    repiota = nc.dram_tensor("repiota", [P, max_k * WN], F32, kind="ExternalInput")
    out = nc.dram_tensor("out", [NODES_PAD, D], F32, kind="ExternalOutput")

    out_v = out.ap().rearrange("(w p) d -> p w d", p=P)

    with tile.TileContext(nc) as tc:
        with (
            tc.tile_pool(name="const", bufs=1) as cpool,
            tc.tile_pool(name="seg0", bufs=2) as seg0p,
            tc.tile_pool(name="seg1", bufs=2) as seg1p,
            tc.tile_pool(name="seg2", bufs=2) as seg2p,
            tc.tile_pool(name="seg3", bufs=2) as seg3p,
            tc.tile_pool(name="oh", bufs=10) as ohp,
            tc.tile_pool(name="msg", bufs=2 * SBG + 2) as msgp,
            tc.tile_pool(name="eps", bufs=2) as epsp,
            tc.tile_pool(name="outs", bufs=3) as outsp,
            tc.tile_pool(name="agg", bufs=2, space="PSUM") as aggp,
            tc.tile_pool(name="gep", bufs=2, space="PSUM") as gepp,
            tc.tile_pool(name="zp", bufs=2, space="PSUM") as zpp,
        ):
            segps = [seg0p, seg1p, seg2p, seg3p]

            gidx_s = cpool.tile([P, tot_slots // 16], I16, tag="gidx")
            nc.sync.dma_start(out=gidx_s[:], in_=gidx.ap())
            scg_s = cpool.tile([P, tot_cc], F32, tag="scg")
            nc.sync.dma_start(out=scg_s[:], in_=scg.ap())
            inv_s = cpool.tile([P, WINDOWS], F32, tag="invdeg")
            nc.sync.dma_start(out=inv_s[:], in_=invdeg.ap())
            wgc_s = cpool.tile([D, 3], F32, tag="wgc")
            nc.sync.dma_start(out=wgc_s[:], in_=wgc.ap())
            wlt_s = cpool.tile([D, D], F32, tag="wlint")
            nc.sync.dma_start(out=wlt_s[:], in_=wlint.ap())
            bgc_s = cpool.tile([P, 3 * WPB], F32, tag="bgc")
            nc.sync.dma_start(out=bgc_s[:], in_=bgc_rep.ap())
            blin_s = cpool.tile([P, D * SBG], F32, tag="blin")
            nc.sync.dma_start(out=blin_s[:], in_=blin_rep.ap())
            ri_s = cpool.tile([P, max_k * WN], F32, tag="repiota")
            nc.sync.dma_start(out=ri_s[:], in_=repiota.ap())

            def epilogue(sb, msgs):
                # ---- epilogue (identical to v3) ----
                inv_sb = inv_s[:, sb * WPB : (sb + 1) * WPB]
                inv_b = inv_sb.rearrange("p (w o) -> p w o", o=1).to_broadcast(
                    [P, WPB, 3]
                )
                gp = gepp.tile([P, 3 * WPB], F32, tag="gep")
                for w in range(WPB):
                    j, wi = w // 2, w % 2
                    nc.tensor.matmul(
                        out=gp[:, 3 * w : 3 * w + 3],
                        lhsT=msgs[j][:, wi * P : (wi + 1) * P],
                        rhs=wgc_s[:],
                        start=True,
                        stop=True,
                    )
                ge_a = epsp.tile([P, 3 * WPB], F32, tag="gea")
                gp3 = gp[:].rearrange("p (w g) -> p w g", g=3)
                nc.vector.tensor_tensor(
                    out=ge_a[:].rearrange("p (w g) -> p w g", g=3),
                    in0=gp3, in1=inv_b, op=mybir.AluOpType.mult,
                )
                ge_b = epsp.tile([P, 3 * WPB], F32, tag="geb")
                nc.vector.tensor_tensor(
                    out=ge_b[:], in0=ge_a[:], in1=bgc_s[:], op=mybir.AluOpType.add
                )
                ge_s = epsp.tile([P, 3 * WPB], F32, tag="ge")
                nc.vector.tensor_scalar(
                    out=ge_s[:],
                    in0=ge_b[:],
                    scalar1=0.0,
                    scalar2=None,
                    op0=mybir.AluOpType.max,
                )
                ge3 = ge_s[:].rearrange("p (w g) -> p w g", g=3)
                top = epsp.tile([P, WPB], F32, tag="top")
                nc.vector.tensor_reduce(
                    out=top[:],
                    in_=ge3,
                    axis=mybir.AxisListType.X,
                    op=mybir.AluOpType.max,
                )
                mask = epsp.tile([P, 3 * WPB], F32, tag="mask")
                top_b = top[:].rearrange("p (w o) -> p w o", o=1).to_broadcast(
                    [P, WPB, 3]
                )
                nc.vector.tensor_tensor(
                    out=mask[:].rearrange("p (w g) -> p w g", g=3),
                    in0=ge3,
                    in1=top_b,
                    op=mybir.AluOpType.is_equal,
                )
                mult_t = epsp.tile([P, WPB], F32, tag="mult")
                nc.vector.tensor_reduce(
                    out=mult_t[:],
                    in_=mask[:].rearrange("p (w g) -> p w g", g=3),
                    axis=mybir.AxisListType.X,
                    op=mybir.AluOpType.add,
                )
                q = epsp.tile([P, WPB], F32, tag="q")
                nc.vector.tensor_tensor(
                    out=q[:], in0=mult_t[:], in1=inv_sb, op=mybir.AluOpType.mult
                )

                for half in range(2):
                    zp = zpp.tile([P, D * SBG], F32, tag="zp")
                    for k in range(SBG):
                        w = half * SBG + k
                        j, wi = w // 2, w % 2
                        nc.tensor.matmul(
                            out=zp[:, k * D : (k + 1) * D],
                            lhsT=msgs[j][:, wi * P : (wi + 1) * P],
                            rhs=wlt_s[:],
                            start=True,
                            stop=True,
                        )
                    os_ = outsp.tile([P, D * SBG], F32, tag="outs")
                    qh = (
                        q[:, half * SBG : (half + 1) * SBG]
                        .rearrange("p (w o) -> p w o", o=1)
                        .to_broadcast([P, SBG, D])
                    )
                    nc.vector.tensor_tensor(
                        out=os_[:].rearrange("p (w d) -> p w d", d=D),
                        in0=zp[:].rearrange("p (w d) -> p w d", d=D),
                        in1=qh,
                        op=mybir.AluOpType.mult,
                    )
                    os2 = outsp.tile([P, D * SBG], F32, tag="outs2")
                    nc.vector.tensor_tensor(
                        out=os2[:], in0=os_[:], in1=blin_s[:], op=mybir.AluOpType.add
                    )
                    w0 = sb * WPB + half * SBG
                    nc.sync.dma_start(
                        out=out_v[:, w0 : w0 + SBG, :],
                        in_=os2[:].rearrange("p (w d) -> p w d", d=D),
                    )


            pending = []
            for _rep in range(n_reps):
                for sb in range(SB):
                    segs = []
                    for r in range(NR):
                        ncols = L["seg_cols"][sb][r]
                        nslots = ncols * P
                        slot_off = L["seg_slot_off"][sb][r]
                        seg = segps[r].tile([P, ncols, D], F32, tag=f"seg{r}")
                        nc.gpsimd.dma_gather(
                            out_ap=seg[:],
                            in_ap=feat.ap()[
                                RANGE_STARTS[r] : RANGE_STARTS[r] + RANGE_ROWS[r], :
                            ],
                            idxs_ap=gidx_s[:, slot_off // 16 : (slot_off + nslots) // 16],
                            num_idxs=nslots,
                            num_idxs_reg=nslots,
                            elem_size=D,
                            single_packet=False,
                            queue_num=qassign[(sb, r)],
                        )
                        segs.append(seg)

                    msgs = []
                    for j in range(SBG):
                        jg = sb * SBG + j
                        # side-masked one-hot chunk per (half, range)
                        ohs = {}
                        for h in range(2):
                            for r in range(NR):
                                c0, nck, cc0 = chunk_of[(jg, h, r)]
                                oh = ohp.tile([P, max_k * WN], F32, tag="oh")
                                scb = (
                                    scg_s[:, cc0 : cc0 + nck]
                                    .rearrange("p (k o) -> p k o", o=1)
                                    .to_broadcast([P, nck, WN])
                                )
                                nc.vector.tensor_tensor(
                                    out=oh[:, : nck * WN].rearrange(
                                        "p (k w) -> p k w", w=WN
                                    ),
                                    in0=scb,
                                    in1=ri_s[:, : nck * WN].rearrange(
                                        "p (k w) -> p k w", w=WN
                                    ),
                                    op=mybir.AluOpType.is_equal,
                                )
                                ohs[(h, r)] = (oh, c0, nck)

                        psum = aggp.tile([D, GW], F32, tag="agg")
                        for h in range(2):
                            parts = []
                            for r in range(NR):
                                oh, c0, nck = ohs[(h, r)]
                                for k in range(nck):
                                    parts.append((r, c0 + k, oh, k))
                            nmm = len(parts)
                            for ci, (r, segc, oh, ohk) in enumerate(parts):
                                nc.tensor.matmul(
                                    out=psum[:, h * WN : (h + 1) * WN],
                                    lhsT=segs[r][:, segc, :],
                                    rhs=oh[:, ohk * WN : (ohk + 1) * WN],
                                    start=(ci == 0),
                                    stop=(ci == nmm - 1),
                                )
                        msgT = msgp.tile([D, GW], F32, tag="msg")
                        nc.scalar.copy(out=msgT[:], in_=psum[:])
                        msgs.append(msgT)

                    pending.append((sb, msgs))
                    if len(pending) > 1:
                        epilogue(*pending.pop(0))
                for sbv, msgsv in pending:
                    epilogue(sbv, msgsv)
                pending = []
    nc.compile()
    return nc


def host_prep(feature, src, dst, W_gc, b_gc, W_lin, b_lin):
    src = np.asarray(src).astype(np.int64)
    dst = np.asarray(dst).astype(np.int64)
    feat32 = np.ascontiguousarray(np.asarray(feature, dtype=np.float32))

    core_of = dst // NODES_PER_CORE
    nloc_all = dst - core_of * NODES_PER_CORE

    deg = np.bincount(dst, minlength=N_NODES).astype(np.float32)
    invd = np.where(deg > 0, 1.0 / np.maximum(deg, 1.0), 0.0).astype(np.float32)

    r_all = (src // RANGE).astype(np.int64)
    grp_all = nloc_all // GW
    half_all = (nloc_all % GW) // WN
    NB = GROUPS * 2 * NR
    cnt_max = np.zeros(NB, dtype=np.int64)
    per_core = []
    for c in range(N_CORES):
        m = core_of == c
        e_src = src[m]
        e_nloc = nloc_all[m]
        e_r = r_all[m]
        e_grp = grp_all[m]
        e_half = half_all[m]
        key = (e_grp * 2 + e_half) * NR + e_r
        cnt = np.bincount(key, minlength=NB)
        cnt_max = np.maximum(cnt_max, cnt)
        per_core.append((e_src, e_nloc, e_r, e_grp, e_half, key))

    caps = np.maximum(Q32, -(-cnt_max // Q32) * Q32).reshape(GROUPS, 2, NR)
    L = _layout(caps)
    tot_slots = L["tot_slots"]
    tot_cc = L["tot_chunk_cols"]
    max_k = L["max_chunk_cols"]
    bucket_slot = L["bucket_slot"].reshape(-1)  # by key (jg*2+h)*NR + r

    repiota = np.tile(np.arange(WN, dtype=np.float32), (P, max_k)).reshape(
        P, max_k * WN
    )
    wgc = np.ascontiguousarray(np.asarray(W_gc, dtype=np.float32))
    wlint = np.ascontiguousarray(np.asarray(W_lin, dtype=np.float32).T)
    bgc_rep = np.tile(np.asarray(b_gc, dtype=np.float32).reshape(1, 3), (P, WPB))
    blin_rep = np.tile(np.asarray(b_lin, dtype=np.float32).reshape(1, D), (P, SBG))

    caps_flat = caps.reshape(-1)

    in_maps = []
    for c in range(N_CORES):
        e_src, e_nloc, e_r, e_grp, e_half, key = per_core[c]

        order = np.argsort(key, kind="stable")
        k_sorted = key[order]
        start_of = np.zeros(NB, dtype=np.int64)
        start_of[1:] = np.cumsum(np.bincount(k_sorted, minlength=NB))[:-1]
        rank = np.arange(k_sorted.size) - start_of[k_sorted]

        slot = bucket_slot[k_sorted] + rank
        assert (rank < caps_flat[k_sorted]).all(), "bucket cap exceeded"

        pad_rows = min(RANGE_ROWS)
        gidx_flat = ((np.arange(tot_slots, dtype=np.int64) * 7) % pad_rows).astype(
            np.int16
        )
        gidx_flat[slot] = (e_src[order] - e_r[order] * RANGE).astype(np.int16)
        scv = np.full(tot_slots, -1000.0, dtype=np.float32)
        scv[slot] = (e_nloc[order] % WN).astype(np.float32)

        # side-masked chunk sc: for each chunk column, only the owning
        # bucket's slots keep their value; foreign slots get the sentinel
        scg_arr = np.full((P, tot_cc), -1000.0, dtype=np.float32)
        for jg, h, r, c0, nck, cc0 in L["chunks"]:
            b = (jg * 2 + h) * NR + r
            b0 = int(bucket_slot[b])
            cap = int(caps_flat[b])
            sb = jg // SBG
            seg0 = L["seg_slot_off"][sb][r]
            for k in range(nck):
                gslot0 = seg0 + (c0 + k) * P  # column's global slot base
                lo = max(b0, gslot0)
                hi = min(b0 + cap, gslot0 + P)
                if lo < hi:
                    scg_arr[lo - gslot0 : hi - gslot0, cc0 + k] = scv[lo:hi]

        wrapped = gidx_flat.reshape(-1, 16).T
        gidx_w = np.tile(wrapped, (8, 1)).astype(np.int16)

        iv = np.zeros(NODES_PAD, dtype=np.float32)
        iv[:NODES_PER_CORE] = invd[c * NODES_PER_CORE : (c + 1) * NODES_PER_CORE]
        invdeg_c = np.ascontiguousarray(iv.reshape(WINDOWS, P).T)

        in_maps.append(
            {
                "feat32": feat32,
                "gidx": gidx_w,
                "scg": scg_arr,
                "invdeg": invdeg_c,
                "wgc": wgc,
                "wlint": wlint,
                "bgc_rep": bgc_rep,
                "blin_rep": blin_rep,
                "repiota": repiota,
            }
        )

    return in_maps, tuple(caps.reshape(-1).tolist())


_PROGRAM_CACHE = {}


def kernel(**inputs):
    feature = inputs["feature"]
    src = inputs["src"]
    dst = inputs["dst"]
    in_maps, caps = host_prep(
        feature,
        src,
        dst,
        inputs["W_gc"],
        inputs["b_gc"],
        inputs["W_lin"],
        inputs["b_lin"],
    )
    if caps not in _PROGRAM_CACHE:
        _PROGRAM_CACHE[caps] = build_program(np.asarray(caps))
    nc = _PROGRAM_CACHE[caps]
    res = bass_utils.run_bass_kernel_spmd(nc, in_maps, core_ids=list(range(N_CORES)))
    out = np.concatenate(
        [res.results[c]["out"][:NODES_PER_CORE] for c in range(N_CORES)], axis=0
    )
    return out.astype(np.float32)
